# revision 1
# baseline (speedup 1.0000x reference)
"""Block-sparse (top-2 block) attention for TRN2, sharded over 8 NeuronCores.

Problem: q [1,8,2048,64], k/v [1,8,4096,64], top2_idx [1,8,2048,2] over 64
key-blocks of 64 rows. Per query: softmax over the 128 keys of its 2 selected
blocks, weighted sum of V.

Strategy (per core = one head), v2 — transposed-gather dataflow:
  - Route (query, slot) pairs into per-block groups of capacity CAP. Each
    block attends densely: its routed queries vs its 64 keys.
  - Q and K are converted to fp16 (row-duplicated to fill 256B elems) and
    pulled into dk-major "Qt"/"Kt" layout by SWDGE transposed gathers with
    SBUF source -- no PE transposes, no DRAM round trip. K gathers 512B
    row-pairs (halves descriptors, avoids the <512B DMA penalty); the
    resulting even/odd key interleave is mirrored in the V load.
  - Per block: one fp16 matmul -> scores^T in PSUM; ACT exp (scale 1/8, no
    max-subtraction: scores ~N(0,1)); fp16 matmul against [V | 1] gives
    numerator rows + softmax denominator. Partials land in SBUF as bf16.
  - A third transposed SBUF-source gather brings each query's two partial
    rows side by side in dk-major layout; DVE adds them; 16 PE transposes
    flip back to query-major (ref ordering chosen so the final store is
    contiguous); DVE normalizes by the gathered denominator.
Routing indices are computed on host (tiny argsort per head) and passed as
int16 index tensors in the dma_gather wire format.
"""

import numpy as np

import concourse.bass as bass
import concourse.bacc as bacc
import concourse.mybir as mybir
import concourse.tile as tile
from concourse.masks import make_identity

F32 = mybir.dt.float32
F16 = mybir.dt.float16
BF16 = mybir.dt.bfloat16
I16 = mybir.dt.int16

T_Q = 2048
T_K = 4096
DK = 64
BS = 64
NB = T_K // BS        # 64 key blocks
NR = 2 * T_Q          # 4096 real (query, slot) refs
N_CORES = 8
DEF_CAP = 88          # routed-query capacity per block (counts.max()=87 at seed 0)

EXP = mybir.ActivationFunctionType.Exp
ADD = mybir.AluOpType.add
MULT = mybir.AluOpType.mult


def build_module(reps: int = 1, variant: str = "full",
                 gather_queues: int = 4, cap: int = DEF_CAP) -> bass.Bass:
    """reps>1 repeats the whole body back-to-back under tc.For_i (for
    wall-clock slope timing of the steady-state per-iteration cost)."""
    NG = NB * cap
    nc = bacc.Bacc("TRN2", target_bir_lowering=False, debug=False,
                   num_swdge_queues=max(1, gather_queues),
                   dynamic_dma_scratch_size=32768)
    q = nc.dram_tensor("q", [T_Q, DK], F32, kind="ExternalInput")
    k = nc.dram_tensor("k", [T_K, DK], F32, kind="ExternalInput")
    v = nc.dram_tensor("v", [T_K, DK], F32, kind="ExternalInput")
    qg_idx = nc.dram_tensor("qg_idx", [128, NG // 16], I16, kind="ExternalInput")
    kg_idx = nc.dram_tensor("kg_idx", [128, (T_K // 2) // 16], I16,
                            kind="ExternalInput")
    og_idx = nc.dram_tensor("og_idx", [128, NR // 16], I16, kind="ExternalInput")
    out = nc.dram_tensor("out", [T_Q, DK], F32, kind="ExternalOutput")

    with tile.TileContext(nc) as tc:
        with (
            tc.tile_pool(name="const", bufs=1) as constp,
            tc.tile_pool(name="big", bufs=1) as bigp,
            tc.tile_pool(name="work", bufs=4) as workp,
            tc.tile_pool(name="psum_s", bufs=2, space="PSUM") as psumsp,
            tc.tile_pool(name="psum_o", bufs=2, space="PSUM") as psumop,
            tc.tile_pool(name="psum_t", bufs=2, space="PSUM") as psumtp,
        ):
            ident = constp.tile([128, 128], F32)
            make_identity(nc, ident[:])

            qgi = constp.tile([128, NG // 16], I16)
            nc.sync.dma_start(out=qgi[:], in_=qg_idx[:])
            kgi = constp.tile([128, (T_K // 2) // 16], I16)
            nc.sync.dma_start(out=kgi[:], in_=kg_idx[:])
            ogi = constp.tile([128, NR // 16], I16)
            nc.sync.dma_start(out=ogi[:], in_=og_idx[:])

            # obig pad columns are zero-initialized once (on the otherwise-idle
            # GPSIMD engine); the loop body only ever writes cols 0:W.
            obig = constp.tile([cap, NB, 128], BF16, tag="obig", name="obig")
            nc.gpsimd.memset(obig[:, :, DK + 1:128], 0.0)

            def body():
                _emit_body(nc, tc, q, k, v, out, ident, qgi, kgi, ogi, obig,
                           bigp, workp, psumsp, psumop, psumtp,
                           cap, gather_queues)

            if reps == 1:
                body()
            else:
                with tc.For_i(0, reps, 1):
                    body()

    nc.compile()
    return nc


def _emit_body(nc, tc, q, k, v, out, ident, qgi, kgi, ogi, obig,
               bigp, workp, psumsp, psumop, psumtp, cap, gather_queues,
               dump=None):
    NG = NB * cap
    NKP = T_K // 2            # 2048 key pairs

    # ---- loads (q then k) and fp16 row-duplicated converts ----
    qf = bigp.tile([128, 16, DK], F32, tag="qf")
    nc.sync.dma_start(
        out=qf[:], in_=q[:].rearrange("(p c) d -> p c d", p=128)
    )
    kf = bigp.tile([128, 32, DK], F32, tag="kf")
    nc.sync.dma_start(
        out=kf[:], in_=k[:].rearrange("(p c) d -> p c d", p=128)
    )
    qd = bigp.tile([128, 16, 128], F16, tag="qd")
    nc.vector.tensor_copy(out=qd[:, :, 0:DK], in_=qf[:])
    nc.vector.tensor_copy(out=qd[:, :, DK:128], in_=qf[:])
    kd = bigp.tile([128, 32, 128], F16, tag="kd")
    nc.vector.tensor_copy(out=kd[:, :, 0:DK], in_=kf[:])
    nc.vector.tensor_copy(out=kd[:, :, DK:128], in_=kf[:])

    # ---- transposed SBUF-source gathers: Qt [128, NG], Kt2 [128, 2, NKP].
    # HW constraint (found empirically): a transposed gather must not begin
    # (its gen self-programs the shared transpose-RX config) while another
    # transposed transfer is in flight. The three gathers are serialized by
    # pure dataflow: each one's index tile is staged through a copy that has
    # a WAW dependency on a probe-read of the previous gather's output. ----
    qt = bigp.tile([128, NG], F16, tag="qt", name="qt")
    kt = bigp.tile([128, 2, NKP], F16, tag="kt", name="kt")
    otg = bigp.tile([128, NR], BF16, tag="ot", name="ot")

    nc.gpsimd.dma_gather(
        out_ap=qt[:, 0:NG // 2].rearrange("p (o n) -> p o n", o=1),
        in_ap=qd[:],
        idxs_ap=qgi[:, 0:NG // 32],
        num_idxs=NG // 2,
        num_idxs_reg=NG // 2,
        elem_size=128,
        transpose=True,
        single_packet=False,
        queue_num=0,
        sbuf_tokens_per_rank=128,
        sbuf_free_dim_per_rank=256,
    )
    # kt after qt0: stage kt idxs behind a probe of qt half 0
    kstage = bigp.tile([128, NKP // 16], I16, tag="kstage")
    nc.vector.tensor_copy(out=kstage[:, 0:1], in_=qt[:, 0:1])
    nc.vector.tensor_copy(out=kstage[:], in_=kgi[:])
    nc.gpsimd.dma_gather(
        out_ap=kt[:],
        in_ap=kd[:],
        idxs_ap=kstage[:],
        num_idxs=NKP,
        num_idxs_reg=NKP,
        elem_size=256,
        transpose=True,
        single_packet=False,
        queue_num=2,
        sbuf_tokens_per_rank=128,
        sbuf_free_dim_per_rank=512,
    )
    # qt1 after kt: stage its idxs behind a probe of kt
    qstage = bigp.tile([128, NG // 32], I16, tag="qstage")
    nc.vector.tensor_copy(out=qstage[:, 0:1], in_=kt[:, 0, 0:1])
    nc.vector.tensor_copy(out=qstage[:], in_=qgi[:, NG // 32:NG // 16])
    nc.gpsimd.dma_gather(
        out_ap=qt[:, NG // 2:NG].rearrange("p (o n) -> p o n", o=1),
        in_ap=qd[:],
        idxs_ap=qstage[:],
        num_idxs=NG // 2,
        num_idxs_reg=NG // 2,
        elem_size=128,
        transpose=True,
        single_packet=False,
        queue_num=1,
        sbuf_tokens_per_rank=128,
        sbuf_free_dim_per_rank=256,
    )

    # De-interleave the pair-gathered Kt into natural key order (walrus
    # requires single-free-dim matmul weight APs): ktc col 2c+f <- kt[:, f, c]
    ktc = bigp.tile([128, T_K], F16, tag="ktc", name="ktc")
    for f in range(2):
        nc.vector.tensor_copy(
            out=ktc[:].rearrange("p (c f) -> p f c", f=2)[:, f, :],
            in_=kt[:, f, :],
        )

    # ---- V in pair-chunk layout: block 2m at partitions 0:64 of chunk m,
    # block 2m+1 at partitions 64:128 (quadrant scheme). Chunk DMAs are
    # chained behind gather triggers so their (safe, HWDGE) transfers slot
    # into the DMA stream after the gather transfers ahead of them. ----
    vf = bigp.tile([128, NB // 2, DK], F32, tag="vf")
    vhat = bigp.tile([128, NB // 2, DK + 1], F16, tag="vhat")
    nc.vector.memset(vhat[:, :, DK:DK + 1], 1.0)
    for h in range(2):
        nc.sync.dma_start(
            out=vf[:, (NB // 4) * h:(NB // 4) * (h + 1), :],
            in_=v[:].rearrange("(m p) d -> p m d", p=128)[
                :, (NB // 4) * h:(NB // 4) * (h + 1), :],
        )
        nc.vector.tensor_copy(
            out=vhat[:, (NB // 4) * h:(NB // 4) * (h + 1), 0:DK],
            in_=vf[:, (NB // 4) * h:(NB // 4) * (h + 1), :],
        )

    # ---- per 4-pair group: scores (even blocks on partitions 0:64, odd on
    # 64:128 via the dup rows of Qt/Kt) -> exp -> [o | Z] partials -> obig ----
    W = DK + 1
    for g in range(NB // 8):
        s_ps = psumsp.tile([128, 4 * cap], F32, tag="s_ps")
        for bm in range(4):
            m = 4 * g + bm                 # pair index; blocks 2m, 2m+1
            for par in range(2):
                j = 2 * m + par
                b = 64 * par
                nc.tensor.matmul(
                    s_ps[b:b + 64, bm * cap:(bm + 1) * cap],
                    lhsT=ktc[b:b + 64, BS * j:BS * (j + 1)],
                    rhs=qt[b:b + 64, cap * j:cap * (j + 1)],
                    start=True, stop=True,
                )
        e_sb = workp.tile([128, 4 * cap], F16, tag="e_sb")
        nc.scalar.activation(
            out=e_sb[:], in_=s_ps[:], func=EXP, scale=1.0 / np.sqrt(DK)
        )
        # HW restriction: >2 matmuls with mixed operand partition-bases into
        # one PSUM tile fail at runtime -> separate tiles per parity.
        for half in range(2):
            o_ev = psumop.tile([cap, 2 * W], F32, tag="o_ev")
            o_od = psumop.tile([cap, 2 * W], F32, tag="o_od")
            for pj in range(4):
                c = pj // 2
                m = 4 * g + 2 * half + c
                par = pj % 2
                b = 64 * par
                t = o_ev if par == 0 else o_od
                nc.tensor.matmul(
                    t[:, c * W:(c + 1) * W],
                    lhsT=e_sb[b:b + 64,
                              (2 * half + c) * cap:(2 * half + c + 1) * cap],
                    rhs=vhat[b:b + 64, m, :],
                    start=True, stop=True,
                )
            # even strips c -> block 8g+4half+2c (u=0), odd -> +2c+1 (u=1)
            ob4 = obig[:].rearrange("p (j u) x -> p j u x", u=2)
            j2 = 4 * g + 2 * half
            nc.scalar.copy(
                out=ob4[:, j2:j2 + 2, 0, 0:W],
                in_=o_ev[:].rearrange("p (c x) -> p c x", c=2),
            )
            nc.vector.tensor_copy(
                out=ob4[:, j2:j2 + 2, 1, 0:W],
                in_=o_od[:].rearrange("p (c x) -> p c x", c=2),
            )

    # ---- gather partial rows (dk-major) and finish, in two independent
    # query-parity pipelines; descriptors were prepped above so only the
    # transfers remain after the last partial lands ----
    if dump is not None:
        dump.update(qt=qt, kt=kt, ktc=ktc, vhat=vhat, ot=otg)
    # og as one plain gather (a single transfer cannot self-conflict).
    nc.gpsimd.dma_gather(
        out_ap=otg[:].rearrange("p (o n) -> p o n", o=1),
        in_ap=obig[:],
        idxs_ap=ogi[:],
        num_idxs=NR,
        num_idxs_reg=NR,
        elem_size=128,
        transpose=True,
        single_packet=False,
        queue_num=3,
        sbuf_tokens_per_rank=128,
        sbuf_free_dim_per_rank=256,
    )
    for h in range(2):
        osum = bigp.tile([128, T_Q // 2], F32, tag=f"osum{h}")
        nc.vector.tensor_tensor(
            out=osum[:],
            in0=otg[:, T_Q * h:T_Q * h + T_Q // 2],
            in1=otg[:, T_Q * h + T_Q // 2:T_Q * (h + 1)],
            op=ADD,
        )
        outv = bigp.tile([128, 8, DK], F32, tag=f"outv{h}")
        for cg in range(2):
            t_ps = psumtp.tile([128, 512], F32, tag="t_ps")
            for c in range(4):
                nc.tensor.transpose(
                    out=t_ps[:, c * 128:(c + 1) * 128],
                    in_=osum[:, (4 * cg + c) * 128:(4 * cg + c + 1) * 128],
                    identity=ident[:],
                )
            zr = workp.tile([128, 4], F32, tag="zr")
            t4 = t_ps[:].rearrange("p (c x) -> p c x", c=4)
            nc.vector.reciprocal(out=zr[:], in_=t4[:, :, DK])
            nc.vector.tensor_tensor(
                out=outv[:, 4 * cg:4 * (cg + 1), :], in0=t4[:, :, 0:DK],
                in1=zr[:, :, None].to_broadcast([128, 4, DK]), op=MULT,
            )
        nc.sync.dma_start(
            out=out[:].rearrange("(p c) d -> p c d", p=128)[:, 8 * h:8 * (h + 1), :],
            in_=outv[:],
        )


_CACHE: dict = {}


def get_module(cap: int = DEF_CAP) -> bass.Bass:
    key = ("m", cap)
    if key not in _CACHE:
        _CACHE[key] = build_module(gather_queues=4, cap=cap)
    return _CACHE[key]


def _wire(vals: np.ndarray, n: int) -> np.ndarray:
    """int16 idx wire format: flat position i at [i%16, i//16], replicated to
    all 8 GPSIMD core groups."""
    w = np.empty((128, n // 16), np.int16)
    w[:] = np.tile(vals.astype(np.int16).reshape(n // 16, 16).T, (8, 1))
    return w


def routing(idx2: np.ndarray, cap: int = DEF_CAP):
    """idx2: [2048, 2] int32 block ids -> (qg, kg, og) int16 wire tensors.

    Token spaces (SBUF-source transposed gathers, tpr=128):
      q row i       -> tau = (i%16)*128 + i//16        (qd: 16 ranks of 256B)
      key-pair kp   -> tau = (kp%16)*128 + kp//16      (kd: 16 ranks of 512B)
      slot (j,lane) -> tau = j*128 + lane              (obig: 64 ranks of 256B)
    """
    NG = NB * cap
    blocks = np.concatenate([idx2[:, 0], idx2[:, 1]]).astype(np.int64)
    counts = np.bincount(blocks, minlength=NB)
    if counts.max() > cap:
        raise ValueError(f"block over capacity: {counts.max()} > {cap}")
    order = np.argsort(blocks, kind="stable")
    starts = np.cumsum(counts) - counts
    within = np.arange(NR) - np.repeat(starts, counts)
    lane = np.empty(NR, np.int64)
    lane[order] = within
    jblk = blocks
    # qg: gather slot s = j*cap + lane reads q row (ref % T_Q)
    qrow = np.zeros(NG, np.int64)            # pad slots gather q row 0
    qrow[jblk * cap + lane] = np.arange(NR) % T_Q
    qg = _wire((qrow % 16) * 128 + qrow // 16, NG)
    # kg: identity over key pairs
    kp = np.arange(T_K // 2, dtype=np.int64)
    kg = _wire((kp % 16) * 128 + kp // 16, T_K // 2)
    # og: two query-parity halves (h = bit3 of t&15). Within half h, query t
    # puts slot-A at column (t%8)*128 + t//16 and slot-B at +1024; value is
    # the slot token j*128 + lane.
    t = np.arange(T_Q, dtype=np.int64)
    h = (t >> 3) & 1
    col = (t % 8) * 128 + (t >> 4)
    og_vals = np.zeros(NR, np.int64)
    og_vals[h * T_Q + col] = jblk[:T_Q] * 128 + lane[:T_Q]
    og_vals[h * T_Q + col + T_Q // 2] = jblk[T_Q:] * 128 + lane[T_Q:]
    og = _wire(og_vals, NR)
    return qg, kg, og


def pick_cap(top2_idx: np.ndarray) -> int:
    mx = 0
    for i in range(top2_idx.shape[1]):
        blocks = np.asarray(top2_idx[0, i]).reshape(-1).astype(np.int64)
        mx = max(mx, int(np.bincount(blocks, minlength=NB).max()))
    for cap in (DEF_CAP, 96, 112, 128):
        if mx <= cap:
            return cap
    raise ValueError(f"max block count {mx} exceeds 128")


def make_in_maps(q, k, v, top2_idx, cap: int = None):
    if cap is None:
        cap = pick_cap(np.asarray(top2_idx))
    in_maps = []
    for i in range(N_CORES):
        qg, kg, og = routing(np.asarray(top2_idx[0, i]), cap)
        in_maps.append({
            "q": np.ascontiguousarray(np.asarray(q[0, i], dtype=np.float32)),
            "k": np.ascontiguousarray(np.asarray(k[0, i], dtype=np.float32)),
            "v": np.ascontiguousarray(np.asarray(v[0, i], dtype=np.float32)),
            "qg_idx": qg,
            "kg_idx": kg,
            "og_idx": og,
        })
    return in_maps


def kernel(**inputs) -> np.ndarray:
    q = np.asarray(inputs["q"])
    k = np.asarray(inputs["k"])
    v = np.asarray(inputs["v"])
    top2_idx = np.asarray(inputs["top2_idx"])
    assert int(inputs["BS"]) == BS
    assert q.shape == (1, N_CORES, T_Q, DK), q.shape
    assert k.shape == (1, N_CORES, T_K, DK), k.shape

    from concourse.bass_utils import run_bass_kernel_spmd

    cap = pick_cap(top2_idx)
    nc = get_module(cap)
    in_maps = make_in_maps(q, k, v, top2_idx, cap)
    res = run_bass_kernel_spmd(nc, in_maps, list(range(N_CORES)))
    out = np.stack([res.results[i]["out"] for i in range(N_CORES)])
    return out[None].astype(np.float32)



# revision 2
# speedup vs baseline: 3.0060x; 3.0060x over previous
"""Block-sparse top-2 attention, v4: depadded routed-Q gather (per-head baked).

vs v3: the routed-Q DRAM gather drops cap padding (5632 -> 4096 idxs; SWDGE
descriptor generation is ~12.5ns/idx and is the kernel's critical path).
Tokens = the 4096 (query, slot) refs in block-sorted order. Each block j's
scores matmul reads a fixed-width WIN=88 window [win_j, win_j+88) of the dense
token axis (win_j = start_j clamped per gather chunk, baked per head). Window
columns outside the block's real refs compute garbage scores whose PV output
lands in obig lanes that og never reads. obig stays in v3's window-lane space
[WIN, NB, 128]; og token values just shift by d_j = start_j - win_j.

Per-head modules (wins baked into APs) -> 8 single-core NEFFs dispatched
concurrently on the 8 devices.
"""

import numpy as np

import concourse.bass as bass
import concourse.bacc as bacc
import concourse.mybir as mybir
import concourse.tile as tile
from concourse.masks import make_identity


def _io_names(nc):
    in_names, out_names, out_avals = [], [], []
    pname = nc.partition_id_tensor.name if nc.partition_id_tensor else None
    import jax as _jax
    for alloc in nc.m.functions[0].allocations:
        if not isinstance(alloc, mybir.MemoryLocationSet):
            continue
        name = alloc.memorylocations[0].name
        if alloc.kind == "ExternalInput":
            if name != pname:
                in_names.append(name)
        elif alloc.kind == "ExternalOutput":
            out_names.append(name)
            out_avals.append(_jax.core.ShapedArray(
                tuple(alloc.tensor_shape), mybir.dt.np(alloc.dtype)))
    return in_names, out_names, out_avals, pname

F32 = mybir.dt.float32
F16 = mybir.dt.float16
BF16 = mybir.dt.bfloat16
I16 = mybir.dt.int16

T_Q = 2048
T_K = 4096
DK = 64
BS = 64
NB = T_K // BS
NR = 2 * T_Q          # 4096 = dense token count
N_CORES = 8
WIN = 88              # matmul window width (>= max per-block ref count)

EXP = mybir.ActivationFunctionType.Exp
ADD = mybir.AluOpType.add
MULT = mybir.AluOpType.mult


def plan(counts):
    counts = [int(c) for c in counts]
    assert sum(counts) == NR and max(counts) <= WIN
    starts = np.concatenate([[0], np.cumsum(counts)]).astype(int)
    a_end = int(min(NR, -(-starts[NB // 2] // 128) * 128))
    b0 = int((starts[NB // 2] // 128) * 128)
    wins = []
    for j in range(NB):
        chunk_end = a_end if j < NB // 2 else NR
        w = min(int(starts[j]), chunk_end - WIN)
        assert w >= (0 if j < NB // 2 else b0)
        assert starts[j] + counts[j] <= w + WIN
        wins.append(w)
    return dict(counts=counts, starts=[int(s) for s in starts[:NB]],
                wins=wins, a_end=a_end, b0=b0)


def build_module(meta, reps: int = 1) -> bass.Bass:
    a_end, b0 = meta["a_end"], meta["b0"]
    a_len, b_len = a_end, NR - b0

    nc = bacc.Bacc("TRN2", target_bir_lowering=False, debug=False,
                   num_swdge_queues=4, dynamic_dma_scratch_size=32768)
    q = nc.dram_tensor("q", [T_Q, DK], F32, kind="ExternalInput")
    k = nc.dram_tensor("k", [T_K, DK], F32, kind="ExternalInput")
    v = nc.dram_tensor("v", [T_K, DK], F32, kind="ExternalInput")
    qga_idx = nc.dram_tensor("qga_idx", [128, a_len // 16], I16,
                             kind="ExternalInput")
    qgb_idx = nc.dram_tensor("qgb_idx", [128, b_len // 16], I16,
                             kind="ExternalInput")
    og_idx = nc.dram_tensor("og_idx", [128, NR // 16], I16, kind="ExternalInput")
    out = nc.dram_tensor("out", [T_Q, DK], F32, kind="ExternalOutput")

    with tile.TileContext(nc) as tc:
        with (
            tc.tile_pool(name="const", bufs=1) as constp,
            tc.tile_pool(name="big", bufs=1) as bigp,
            tc.tile_pool(name="work", bufs=4) as workp,
            tc.tile_pool(name="psum_s", bufs=2, space="PSUM") as psumsp,
            tc.tile_pool(name="psum_o", bufs=1, space="PSUM") as psumop,
            tc.tile_pool(name="psum_t", bufs=2, space="PSUM") as psumtp,
            tc.tile_pool(name="psum_x", bufs=2, space="PSUM") as psumxp,
        ):
            ident = constp.tile([128, 128], F32)
            make_identity(nc, ident[:])
            identh = constp.tile([128, 128], F16)
            nc.vector.tensor_copy(out=identh[:], in_=ident[:])

            qgai = constp.tile([128, a_len // 16], I16)
            nc.sync.dma_start(out=qgai[:], in_=qga_idx[:])
            qgbi = constp.tile([128, b_len // 16], I16)
            nc.sync.dma_start(out=qgbi[:], in_=qgb_idx[:])
            ogi = constp.tile([128, NR // 16], I16)
            nc.sync.dma_start(out=ogi[:], in_=og_idx[:])

            obig = constp.tile([WIN, NB, 128], BF16, tag="obig", name="obig")
            nc.gpsimd.memset(obig[:, :, DK + 1:128], 0.0)

            def body():
                _emit_body(nc, tc, q, k, v, out, ident, identh,
                           qgai, qgbi, ogi, obig, bigp, workp,
                           psumsp, psumop, psumtp, psumxp, meta)

            if reps == 1:
                body()
            else:
                with tc.For_i(0, reps, 1):
                    body()

    nc.compile()
    return nc


def _emit_body(nc, tc, q, k, v, out, ident, identh, qgai, qgbi, ogi, obig,
               bigp, workp, psumsp, psumop, psumtp, psumxp, meta):
    wins = meta["wins"]
    a_end, b0 = meta["a_end"], meta["b0"]
    a_len, b_len = a_end, NR - b0
    W = DK + 1

    # ---- K path: strided load -> dup f16 -> PE transposes -> ktc ----
    kf2 = bigp.tile([128, 32, DK], F32, tag="kf2")
    nc.sync.dma_start(out=kf2[:], in_=k[:].rearrange("(c p) d -> p c d", p=128))
    kd2 = bigp.tile([128, 32, 128], F16, tag="kd2")
    nc.vector.tensor_copy(out=kd2[:, :, 0:DK], in_=kf2[:])
    nc.vector.tensor_copy(out=kd2[:, :, DK:128], in_=kf2[:])
    ktc = bigp.tile([128, T_K], F16, tag="ktc", name="ktc")
    for c4 in range(8):
        t_ps = psumxp.tile([128, 512], F16, tag="tr_ps")
        for i in range(4):
            nc.tensor.transpose(out=t_ps[:, 128 * i:128 * (i + 1)],
                                in_=kd2[:, 4 * c4 + i, :], identity=identh[:])
        nc.vector.tensor_copy(out=ktc[:, 512 * c4:512 * (c4 + 1)], in_=t_ps[:])

    # ---- V path: quadrant layout ----
    vf = bigp.tile([128, NB // 2, DK], F32, tag="vf")
    vhat = bigp.tile([128, NB // 2, DK + 1], F16, tag="vhat")
    nc.vector.memset(vhat[:, :, DK:DK + 1], 1.0)
    for h in range(2):
        nc.sync.dma_start(
            out=vf[:, (NB // 4) * h:(NB // 4) * (h + 1), :],
            in_=v[:].rearrange("(m p) d -> p m d", p=128)[
                :, (NB // 4) * h:(NB // 4) * (h + 1), :],
        )
        nc.vector.tensor_copy(
            out=vhat[:, (NB // 4) * h:(NB // 4) * (h + 1), 0:DK],
            in_=vf[:, (NB // 4) * h:(NB // 4) * (h + 1), :],
        )

    # ---- Q path: 2 depadded DRAM row-gathers -> dup f16 -> PE transposes ----
    qparts = []
    for g, (idxs, nlen) in enumerate(((qgai, a_len), (qgbi, b_len))):
        nch = nlen // 128
        qg_sb = bigp.tile([128, nch, DK], F32, tag=f"qg_sb{g}")
        qg_d = bigp.tile([128, nch, 128], F16, tag=f"qg_d{g}")
        qth = bigp.tile([128, nlen], F16, tag=f"qt{g}", name=f"qt{g}")
        qparts.append(qth)
        nc.gpsimd.dma_gather(
            out_ap=qg_sb[:], in_ap=q[:], idxs_ap=idxs[:],
            num_idxs=nlen, num_idxs_reg=nlen, elem_size=DK,
            transpose=False, single_packet=False, queue_num=0,
        )
        nc.vector.tensor_copy(out=qg_d[:, :, 0:DK], in_=qg_sb[:])
        nc.vector.tensor_copy(out=qg_d[:, :, DK:128], in_=qg_sb[:])
        for c4 in range(0, nch, 2):
            cw = min(2, nch - c4)
            t_ps = psumxp.tile([128, 512], F16, tag="tr_ps")
            for i in range(cw):
                nc.tensor.transpose(out=t_ps[:, 128 * i:128 * (i + 1)],
                                    in_=qg_d[:, c4 + i, :], identity=identh[:])
            nc.vector.tensor_copy(out=qth[:, 128 * c4:128 * (c4 + cw)],
                                  in_=t_ps[:, 0:128 * cw])

    def qwin(j):
        if j < NB // 2:
            return qparts[0], wins[j]
        return qparts[1], wins[j] - b0

    # ---- mm loop: as v3, rhs = per-block token window ----
    for g in range(NB // 8):
        s_ps = psumsp.tile([128, 4 * WIN], F32, tag="s_ps")
        for bm in range(4):
            m = 4 * g + bm
            for par in range(2):
                j = 2 * m + par
                b = 64 * par
                qth, qoff = qwin(j)
                nc.tensor.matmul(
                    s_ps[b:b + 64, bm * WIN:(bm + 1) * WIN],
                    lhsT=ktc[b:b + 64, BS * j:BS * (j + 1)],
                    rhs=qth[b:b + 64, qoff:qoff + WIN],
                    start=True, stop=True,
                )
        e_sb = workp.tile([128, 4 * WIN], F16, tag="e_sb")
        nc.scalar.activation(
            out=e_sb[:], in_=s_ps[:], func=EXP, scale=1.0 / np.sqrt(DK)
        )
        for half in range(2):
            o_ev = psumop.tile([WIN, 2 * W], F32, tag="o_ev")
            o_od = psumop.tile([WIN, 2 * W], F32, tag="o_od")
            for pj in range(4):
                c = pj // 2
                m = 4 * g + 2 * half + c
                par = pj % 2
                b = 64 * par
                t = o_ev if par == 0 else o_od
                nc.tensor.matmul(
                    t[:, c * W:(c + 1) * W],
                    lhsT=e_sb[b:b + 64,
                              (2 * half + c) * WIN:(2 * half + c + 1) * WIN],
                    rhs=vhat[b:b + 64, m, :],
                    start=True, stop=True,
                )
            ob4 = obig[:].rearrange("p (j u) x -> p j u x", u=2)
            j2 = 4 * g + 2 * half
            nc.scalar.copy(
                out=ob4[:, j2:j2 + 2, 0, 0:W],
                in_=o_ev[:].rearrange("p (c x) -> p c x", c=2),
            )
            nc.vector.tensor_copy(
                out=ob4[:, j2:j2 + 2, 1, 0:W],
                in_=o_od[:].rearrange("p (c x) -> p c x", c=2),
            )

    # ---- og gather + epilogue: as v3 ----
    otg = bigp.tile([128, NR], BF16, tag="ot", name="ot")
    nc.gpsimd.dma_gather(
        out_ap=otg[:].rearrange("p (o n) -> p o n", o=1),
        in_ap=obig[:],
        idxs_ap=ogi[:],
        num_idxs=NR, num_idxs_reg=NR, elem_size=128,
        transpose=True, single_packet=False, queue_num=0,
        sbuf_tokens_per_rank=128, sbuf_free_dim_per_rank=256,
    )
    for h in range(2):
        osum = bigp.tile([128, T_Q // 2], F32, tag=f"osum{h}")
        nc.vector.tensor_tensor(
            out=osum[:],
            in0=otg[:, T_Q * h:T_Q * h + T_Q // 2],
            in1=otg[:, T_Q * h + T_Q // 2:T_Q * (h + 1)],
            op=ADD,
        )
        outv = bigp.tile([128, 8, DK], F32, tag=f"outv{h}")
        for cg in range(2):
            t_ps = psumtp.tile([128, 512], F32, tag="t_ps")
            for c in range(4):
                nc.tensor.transpose(
                    out=t_ps[:, c * 128:(c + 1) * 128],
                    in_=osum[:, (4 * cg + c) * 128:(4 * cg + c + 1) * 128],
                    identity=ident[:],
                )
            zr = workp.tile([128, 4], F32, tag="zr")
            t4 = t_ps[:].rearrange("p (c x) -> p c x", c=4)
            nc.vector.reciprocal(out=zr[:], in_=t4[:, :, DK])
            nc.vector.tensor_tensor(
                out=outv[:, 4 * cg:4 * (cg + 1), :], in0=t4[:, :, 0:DK],
                in1=zr[:, :, None].to_broadcast([128, 4, DK]), op=MULT,
            )
        nc.sync.dma_start(
            out=out[:].rearrange("(p c) d -> p c d", p=128)[:, 8 * h:8 * (h + 1), :],
            in_=outv[:],
        )


def _wire(vals: np.ndarray, n: int) -> np.ndarray:
    w = np.empty((128, n // 16), np.int16)
    w[:] = np.tile(vals.astype(np.int16).reshape(n // 16, 16).T, (8, 1))
    return w


def routing(idx2: np.ndarray):
    """idx2: [2048, 2] -> (meta, qga, qgb, og)."""
    blocks = np.concatenate([idx2[:, 0], idx2[:, 1]]).astype(np.int64)
    counts = np.bincount(blocks, minlength=NB)
    meta = plan(counts)
    order = np.argsort(blocks, kind="stable")
    starts = np.array(meta["starts"], np.int64)
    wins = np.array(meta["wins"], np.int64)
    d = starts - wins                       # window offset of first real lane
    qrow_dense = order % T_Q                # token r -> q row
    within = np.arange(NR) - np.repeat(starts, counts)
    lane = np.empty(NR, np.int64)
    lane[order] = within                    # ref -> within-block position
    jblk = blocks
    wlane = lane + d[jblk]                  # ref -> window lane (= obig row)
    a_end, b0 = meta["a_end"], meta["b0"]
    qga = _wire(qrow_dense[0:a_end], a_end)
    qgb = _wire(qrow_dense[b0:NR], NR - b0)
    t = np.arange(T_Q, dtype=np.int64)
    h = (t >> 3) & 1
    col = (t % 8) * 128 + (t >> 4)
    og_vals = np.zeros(NR, np.int64)
    og_vals[h * T_Q + col] = jblk[:T_Q] * 128 + wlane[:T_Q]
    og_vals[h * T_Q + col + T_Q // 2] = jblk[T_Q:] * 128 + wlane[T_Q:]
    og = _wire(og_vals, NR)
    return meta, qga, qgb, og


_CACHE: dict = {}


def get_module(meta, reps: int = 1) -> bass.Bass:
    key = ("m4", reps, tuple(meta["counts"]))
    if key not in _CACHE:
        _CACHE[key] = build_module(meta, reps=reps)
    return _CACHE[key]


def make_cores(q, k, v, top2_idx, reps: int = 1):
    cores = []
    for i in range(N_CORES):
        meta, qga, qgb, og = routing(np.asarray(top2_idx[0, i]))
        nc = get_module(meta, reps=reps)
        cores.append((nc, {
            "q": np.ascontiguousarray(np.asarray(q[0, i], dtype=np.float32)),
            "k": np.ascontiguousarray(np.asarray(k[0, i], dtype=np.float32)),
            "v": np.ascontiguousarray(np.asarray(v[0, i], dtype=np.float32)),
            "qga_idx": qga,
            "qgb_idx": qgb,
            "og_idx": og,
        }))
    return cores


class CoreRunner:
    """Per-device jitted executables for heterogeneous modules."""

    def __init__(self, cores):
        import jax
        from concourse import bass2jax
        from concourse.bass2jax import _bass_exec_p

        bass2jax.install_neuronx_cc_hook()
        self.jax = jax
        devs = jax.devices()[:len(cores)]
        self.entries = []
        for ci, ((nc, in_map), dev) in enumerate(zip(cores, devs)):
            in_names, out_names, out_avals, pname = _io_names(nc)
            all_in = tuple(in_names + out_names)
            if pname is not None:
                all_in = all_in + (pname,)

            def _body(*args, _nc=nc, _avals=tuple(out_avals),
                      _all_in=all_in, _out=tuple(out_names)):
                return tuple(_bass_exec_p.bind(
                    *args, out_avals=_avals, in_names=_all_in, out_names=_out,
                    lowering_input_output_aliases=(),
                    sim_require_finite=True, sim_require_nnan=True, nc=_nc,
                ))

            args = [jax.device_put(np.asarray(in_map[n]), dev)
                    for n in in_names]
            for av in out_avals:
                args.append(jax.device_put(np.zeros(av.shape, av.dtype), dev))
            if pname is not None:
                args.append(jax.device_put(
                    np.array([[ci]], np.uint32), dev))
            self.entries.append((jax.jit(_body, keep_unused=True), args,
                                 out_names))

    def run(self):
        res = [fn(*args) for fn, args, _ in self.entries]
        self.jax.block_until_ready(res)
        return [{n: np.asarray(r[i]) for i, n in enumerate(names)}
                for r, (_, _, names) in zip(res, self.entries)]

    def time_min(self, n_runs=9, warmup=1):
        import time
        for _ in range(warmup):
            self.run()
        ts = []
        for _ in range(n_runs):
            t0 = time.perf_counter()
            self.run()
            ts.append(time.perf_counter() - t0)
        return min(ts), ts


def kernel(**inputs) -> np.ndarray:
    q = np.asarray(inputs["q"])
    k = np.asarray(inputs["k"])
    v = np.asarray(inputs["v"])
    top2_idx = np.asarray(inputs["top2_idx"])
    assert int(inputs["BS"]) == BS
    assert q.shape == (1, N_CORES, T_Q, DK), q.shape

    cores = make_cores(q, k, v, top2_idx)
    res = CoreRunner(cores).run()
    out = np.stack([res[i]["out"] for i in range(N_CORES)])
    return out[None].astype(np.float32)
